# revision 23
# baseline (speedup 1.0000x reference)
import sys

if "/opt/trn_rl_repo" not in sys.path:
    sys.path.insert(0, "/opt/trn_rl_repo")

import numpy as np

# Problem constants (hardcoded per contract)
B = 256          # total batch
DIM = 1024
EMB = 256
K = 8192
GRID = 8
NCORES = 8
BC = B // NCORES          # images per core = 32
IMG2 = 2 * BC             # interleaved first/last images per core = 64
RROWS = 4 * BC            # x-rows per core = 128
EPS = 1e-12

TRACE = False
LAST_EXEC_NS = None

_CACHE = {}


def _sigma_perm():
    # sigma(c') = 4*(c' % 64) + c'//64 ; channel c' of permuted tensors = original channel sigma(c')
    cp = np.arange(EMB)
    return 4 * (cp % 64) + cp // 64


def _build_nc():
    import concourse.bass as bass
    import concourse.bacc as bacc
    import concourse.mybir as mybir
    from concourse import tile

    f32 = mybir.dt.float32
    f32r = mybir.dt.float32r
    u32 = mybir.dt.uint32
    AF = mybir.ActivationFunctionType

    nc = bacc.Bacc(None, target_bir_lowering=False)

    # ---- DRAM parameters (order defines the BIR signature) ----
    XH = nc.declare_dram_parameter("xh", [8, 128, 8, 512], f32, isOutput=False)   # [chunk, p, kt, n]
    WIN = nc.declare_dram_parameter("win", [128, 8, 256], f32, isOutput=False)    # [p, kt, m]
    BIN = nc.declare_dram_parameter("bin", [128, 2], f32, isOutput=False)
    W1P = nc.declare_dram_parameter("w1p", [128, 4608], f32, isOutput=False)      # [ci_p, cit*2304 + kk*256 + co]
    B1 = nc.declare_dram_parameter("b1", [128, 2], f32, isOutput=False)
    W2P = nc.declare_dram_parameter("w2p", [128, 4608], f32, isOutput=False)
    B2 = nc.declare_dram_parameter("b2", [128, 2], f32, isOutput=False)
    CT2 = nc.declare_dram_parameter("ct2", [2, 128, 8192], f32, isOutput=False)   # [et, p, k] = 2*C.T[perm]
    NC2 = nc.declare_dram_parameter("nc2", [1, 8192], f32, isOutput=False)        # -sum(c^2)
    RTP = nc.declare_dram_parameter("rtp", [128, 2, 128], f32, isOutput=False)    # random_vector.T[perm] shard
    WO4 = nc.declare_dram_parameter("wo4", [64, 4, 1024], f32, isOutput=False)    # [v, u, d]
    BOUT = nc.declare_dram_parameter("bout", [1, 1024], f32, isOutput=False)
    IDN = nc.declare_dram_parameter("idn", [128, 128], f32, isOutput=False)
    DEC = nc.declare_dram_parameter("dec", [128, 1024], f32, isOutput=True)       # rows = 32*j + b_local
    TVAL = nc.declare_dram_parameter("tval", [128, 32], f32, isOutput=True)       # 4 chunks x top-8 values
    TIDX = nc.declare_dram_parameter("tidx", [128, 32], u32, isOutput=True)

    def r_(ap):
        return ap.bitcast(f32r)

    with tile.TileContext(nc) as tc:
        with tc.tile_pool(name="outer", bufs=1) as outer:
            xTp0 = outer.tile([128, 128], f32, name="xTp0")
            xTp1 = outer.tile([128, 128], f32, name="xTp1")
            xTp = [xTp0, xTp1]
            vals_sb = outer.tile([128, 32], f32, name="vals_sb")
            idx_sb = outer.tile([128, 32], u32, name="idx_sb")
            ones_r = outer.tile([1, 128], f32, name="ones_r")    # K=1 lhsT
            ones_f = outer.tile([1, 128], f32, name="ones_f")
            ones_c = outer.tile([128, 1], f32, name="ones_c")    # M=1 lhsT
            # f32r provenance: memset cannot write f32r; DVE copy can.
            nc.vector.memset(ones_f[:], 1.0)
            nc.vector.tensor_copy(r_(ones_r[:]), ones_f[:])
            nc.vector.memset(ones_c[:], 1.0)

            # ================= Phase B/C/D: encode =================
            with (
                tc.tile_pool(name="wpool", bufs=1) as wpool,
                tc.tile_pool(name="xcpool", bufs=3) as xcpool,
                tc.tile_pool(name="pppool", bufs=1) as pppool,
                tc.tile_pool(name="c1pool", bufs=1) as c1pool,
                tc.tile_pool(name="psA", bufs=2, space=bass.MemorySpace.PSUM) as psA,
            ):
                win_sb = wpool.tile([128, 8, 256], f32, name="win_sb")
                bin_sb = wpool.tile([128, 2], f32, name="bin_sb")
                w1p_sb = wpool.tile([128, 4608], f32, name="w1p_sb")
                b1_sb = wpool.tile([128, 2], f32, name="b1_sb")
                w2p_sb = wpool.tile([128, 4608], f32, name="w2p_sb")
                b2_sb = wpool.tile([128, 2], f32, name="b2_sb")
                o2T0 = wpool.tile([128, 256], f32, name="o2T0")
                o2T1 = wpool.tile([128, 256], f32, name="o2T1")
                o2T = [o2T0, o2T1]
                pp0 = pppool.tile([128, 6400], f32, name="pp0")
                pp1 = pppool.tile([128, 6400], f32, name="pp1")
                pp = [pp0, pp1]
                c1T0 = c1pool.tile([128, 1024], f32, name="c1T0")
                c1T1 = c1pool.tile([128, 1024], f32, name="c1T1")
                c1T = [c1T0, c1T1]

                # zero the pad ring via DVE copy from a zeros tile
                # (memset cannot write f32r; DVE copy can)
                zpad = wpool.tile([128, 640], f32, name="zpad")
                nc.vector.memset(zpad[:], 0.0)
                z10 = zpad[:].rearrange("p (i w) -> p i w", i=IMG2, w=10)
                z8 = zpad[:, :512].rearrange("p (i w) -> p i w", i=IMG2, w=8)
                for t in pp:
                    v = t[:].rearrange("p (i h w) -> p i h w", i=IMG2, h=10, w=10)
                    nc.vector.tensor_copy(r_(v[:, :, 0, :]), z10)
                    nc.vector.tensor_copy(r_(v[:, :, 9, :]), z10)
                    nc.vector.tensor_copy(r_(v[:, :, 1:9, 0]), z8)
                    nc.vector.tensor_copy(r_(v[:, :, 1:9, 9]), z8)

                # input DMAs (sync HWDGE ring, FIFO order = priority order)
                nc.sync.dma_start(r_(win_sb[:]), r_(WIN[:]))
                nc.sync.dma_start(bin_sb[:], BIN[:])

                # ---- proj: yT = W_in.T @ xT, written into padded image tiles ----
                for ch in range(8):
                    xc = xcpool.tile([128, 8, 512], f32, name="xc")
                    nc.sync.dma_start(r_(xc[:]), r_(XH[ch]))
                    if ch == 0:
                        nc.sync.dma_start(r_(w1p_sb[:]), r_(W1P[:]))
                        nc.sync.dma_start(b1_sb[:], B1[:])
                    if ch == 1:
                        nc.sync.dma_start(r_(w2p_sb[:]), r_(W2P[:]))
                        nc.sync.dma_start(b2_sb[:], B2[:])
                    for mt in range(2):
                        yt = psA.tile([128, 512], f32, name="yt")
                        for kt in range(8):
                            nc.tensor.matmul(
                                yt[:],
                                r_(win_sb[:, kt, mt * 128:(mt + 1) * 128]),
                                r_(xc[:, kt, :]),
                                start=(kt == 0),
                                stop=(kt == 7),
                            )
                        dst = pp[mt][:].rearrange(
                            "p (i h w) -> p i h w", i=IMG2, h=10, w=10
                        )[:, 8 * ch:8 * ch + 8, 1:9, 1:9]
                        src = yt[:].rearrange("p (i h w) -> p i h w", i=8, h=8, w=8)
                        nc.vector.tensor_scalar_add(
                            r_(dst), src, bin_sb[:, mt:mt + 1]
                        )

                # ---- conv1 (3x3 stride2 pad1) + relu ----
                # padded coords: input row = 2*oh + kh, col = 2*ow + kw
                for eo in range(2):
                    for hf in range(2):
                        c1ps = psA.tile([128, 512], f32, name="c1ps")
                        nmm = 0
                        for cit in range(2):
                            src5 = pp[cit][:].rearrange(
                                "p (i q r s t) -> p i q r s t",
                                i=IMG2, q=5, r=2, s=5, t=2,
                            )
                            for kh in range(3):
                                qs = slice(0, 4) if kh < 2 else slice(1, 5)
                                rr = kh % 2
                                for kw in range(3):
                                    ss = slice(0, 4) if kw < 2 else slice(1, 5)
                                    tt = kw % 2
                                    rhs = src5[:, 32 * hf:32 * hf + 32, qs, rr, ss, tt]
                                    lhsT = w1p_sb[
                                        :, cit * 2304 + (kh * 3 + kw) * 256 + eo * 128:
                                        cit * 2304 + (kh * 3 + kw) * 256 + eo * 128 + 128
                                    ]
                                    nc.tensor.matmul(
                                        c1ps[:], r_(lhsT), r_(rhs),
                                        start=(nmm == 0), stop=(nmm == 17),
                                    )
                                    nmm += 1
                        # fused bias-add + relu on DVE (out must be f32r-typed)
                        nc.vector.tensor_scalar(
                            out=r_(c1T[eo][:, 512 * hf:512 * hf + 512]),
                            in0=c1ps[:],
                            scalar1=b1_sb[:, eo:eo + 1],
                            scalar2=0.0,
                            op0=mybir.AluOpType.add,
                            op1=mybir.AluOpType.max,
                        )

                # ---- conv2 (3x3 stride1 nopad) + bias (output channels sigma-permuted) ----
                for eo in range(2):
                    c2ps = psA.tile([128, 256], f32, name="c2ps")
                    nmm = 0
                    for cit in range(2):
                        src4 = c1T[cit][:].rearrange(
                            "p (i h w) -> p i h w", i=IMG2, h=4, w=4
                        )
                        for kh in range(3):
                            for kw in range(3):
                                rhs = src4[:, :, kh:kh + 2, kw:kw + 2]
                                lhsT = w2p_sb[
                                    :, cit * 2304 + (kh * 3 + kw) * 256 + eo * 128:
                                    cit * 2304 + (kh * 3 + kw) * 256 + eo * 128 + 128
                                ]
                                nc.tensor.matmul(
                                    c2ps[:], r_(lhsT), r_(rhs),
                                    start=(nmm == 0), stop=(nmm == 17),
                                )
                                nmm += 1
                    nc.scalar.activation(
                        o2T[eo][:], c2ps[:], AF.Identity, bias=b2_sb[:, eo:eo + 1]
                    )

                # ---- x = last - first  (odd img2 minus even img2) ----
                for eo in range(2):
                    v = o2T[eo][:].rearrange("p (i w q) -> p i w q", i=BC, w=2, q=4)
                    nc.vector.tensor_sub(
                        r_(xTp[eo][:].rearrange("p (i q) -> p i q", i=BC, q=4)),
                        v[:, :, 1, :],
                        v[:, :, 0, :],
                    )

            # ================= Phase F: distances + top-8 =================
            with (
                tc.tile_pool(name="ctpool", bufs=2) as ctpool,
                tc.tile_pool(name="nc2pool", bufs=1) as nc2pool,
                tc.tile_pool(name="gpool", bufs=2) as gpool,
                tc.tile_pool(name="psF", bufs=2, space=bass.MemorySpace.PSUM) as psF,
            ):
                nc2_sb = nc2pool.tile([1, 8192], f32, name="nc2_sb")
                nc.sync.dma_start(r_(nc2_sb[:]), r_(NC2[:]))
                for c in range(4):
                    ct = ctpool.tile([128, 2, 2048], f32, name="ct")
                    nc.sync.dma_start(r_(ct[:, 0, :]), r_(CT2[0, :, c * 2048:(c + 1) * 2048]))
                    nc.sync.dma_start(r_(ct[:, 1, :]), r_(CT2[1, :, c * 2048:(c + 1) * 2048]))
                    gps = psF.tile([128, 2048], f32, name="gps")
                    for nt in range(4):
                        sl = slice(nt * 512, (nt + 1) * 512)
                        nc.tensor.matmul(
                            gps[:, sl], r_(xTp[0][:]), r_(ct[:, 0, sl]),
                            start=True, stop=False,
                        )
                        nc.tensor.matmul(
                            gps[:, sl], r_(xTp[1][:]), r_(ct[:, 1, sl]),
                            start=False, stop=False,
                        )
                        nc.tensor.matmul(
                            gps[:, sl], r_(ones_r[:]),
                            r_(nc2_sb[:, c * 2048 + nt * 512: c * 2048 + (nt + 1) * 512]),
                            start=False, stop=True,
                        )
                    g_sb = gpool.tile([128, 2048], f32, name="g")
                    nc.scalar.activation(g_sb[:], gps[:], AF.Copy)
                    nc.vector.max(vals_sb[:, 8 * c:8 * c + 8], g_sb[:])
                    nc.vector.max_index(
                        idx_sb[:, 8 * c:8 * c + 8], vals_sb[:, 8 * c:8 * c + 8], g_sb[:]
                    )
                nc.scalar.dma_start(TVAL[:], vals_sb[:])
                nc.scalar.dma_start(TIDX[:], idx_sb[:])

            # ================= Phase G/H: norms, q, decode =================
            with (
                tc.tile_pool(name="tail", bufs=1) as tail,
                tc.tile_pool(name="dspool", bufs=2) as dspool,
                tc.tile_pool(name="psG", bufs=1, space=bass.MemorySpace.PSUM) as psG,
                tc.tile_pool(name="psH", bufs=2, space=bass.MemorySpace.PSUM) as psH,
            ):
                rtp_sb = tail.tile([128, 2, 128], f32, name="rtp_sb")
                wo4_sb = tail.tile([64, 4, 1024], f32, name="wo4_sb")
                bout_sb = tail.tile([1, 1024], f32, name="bout_sb")
                idn_sb = tail.tile([128, 128], f32, name="idn_sb")
                nc.sync.dma_start(rtp_sb[:], RTP[:])
                nc.sync.dma_start(r_(wo4_sb[:]), r_(WO4[:]))
                nc.sync.dma_start(r_(bout_sb[:]), r_(BOUT[:]))
                nc.sync.dma_start(idn_sb[:], IDN[:])

                sq = tail.tile([128, 128], f32, name="sq")
                rsq = tail.tile([128, 128], f32, name="rsq")
                gmax8 = tail.tile([128, 8], f32, name="gmax8")
                x2_sb = tail.tile([1, 128], f32, name="x2_sb")
                nr_sb = tail.tile([1, 128], f32, name="nr_sb")
                nres_sb = tail.tile([1, 128], f32, name="nres_sb")
                rn_sb = tail.tile([1, 128], f32, name="rn_sb")
                inv_sb = tail.tile([1, 128], f32, name="inv_sb")
                s_sb = tail.tile([1, 128], f32, name="s_sb")
                qT0 = tail.tile([128, 128], f32, name="qT0")
                qT1 = tail.tile([128, 128], f32, name="qT1")
                qT = [qT0, qT1]
                qs0 = tail.tile([64, 128], f32, name="qs0")
                qs1 = tail.tile([64, 128], f32, name="qs1")
                qsh = [qs0, qs1]
                tmp = tail.tile([128, 128], f32, name="tmp")

                # x2row = sum_e x^2  -> [1, 128] via ones contraction
                x2ps = psG.tile([1, 128], f32, name="x2ps")
                for eo in range(2):
                    nc.scalar.activation(sq[:], xTp[eo][:], AF.Square)
                    nc.tensor.matmul(
                        x2ps[:], ones_c[:], sq[:], start=(eo == 0), stop=(eo == 1)
                    )
                # rnorm2 = sum_d rv^2 -> [1, 128]
                rnps = psG.tile([1, 128], f32, name="rnps")
                for eo in range(2):
                    nc.scalar.activation(rsq[:], rtp_sb[:, eo, :], AF.Square)
                    nc.tensor.matmul(
                        rnps[:], ones_c[:], rsq[:], start=(eo == 0), stop=(eo == 1)
                    )
                # gmax over the 4 chunks' top-8s -> [128,1] -> transpose -> [1,128]
                nc.vector.max(gmax8[:], vals_sb[:])
                gmT = psG.tile([1, 128], f32, name="gmT")
                nc.tensor.transpose(gmT[:], gmax8[:, 0:1], idn_sb[:])

                nc.scalar.activation(x2_sb[:], x2ps[:], AF.Copy)
                nc.vector.tensor_sub(nr_sb[:], x2_sb[:], gmT[:])
                nc.scalar.activation(nres_sb[:], nr_sb[:], AF.Sqrt)
                nc.scalar.activation(rn_sb[:], rnps[:], AF.Sqrt)
                nc.vector.reciprocal(inv_sb[:], rn_sb[:])
                nc.vector.tensor_mul(r_(s_sb[:]), nres_sb[:], inv_sb[:])

                # broadcast s over partitions: sbc[m, r] = s[r]
                sbc = psG.tile([128, 128], f32, name="sbc")
                nc.tensor.matmul(
                    sbc[:], r_(ones_r[:]), r_(s_sb[:]), start=True, stop=True
                )

                # qT = xT + s * rT   (channel-permuted layout)
                for eo in range(2):
                    nc.vector.tensor_mul(tmp[:], rtp_sb[:, eo, :], sbc[:])
                    nc.vector.tensor_add(r_(qT[eo][:]), tmp[:], xTp[eo][:])
                    # shift upper half to base partition 0 for dec lhsT
                    nc.sync.dma_start(r_(qsh[eo][:]), r_(qT[eo][64:128, :]))

                # ---- decode: dec[32j+b, d] = sum_u sum_v q[4b+u, 4v+j] Wout[64u+v, d] + bout
                # per-(chd,j) PSUM tile at base partition 0 (nonzero PE
                # tile_position fails walrus ISA checks), DMA'd PSUM->DRAM
                for chd in range(2):
                    for j in range(4):
                        dps = psH.tile([32, 512], f32, name="dps")
                        nc.tensor.matmul(
                            dps[:], r_(ones_r[:, 0:32]),
                            r_(bout_sb[:, chd * 512:(chd + 1) * 512]),
                            start=True, stop=False,
                        )
                        if j % 2 == 0:
                            qsrc = qT[j // 2][0:64, :]
                        else:
                            qsrc = qsh[j // 2][:]
                        qv = qsrc.rearrange("p (v q) -> p v q", v=32, q=4)
                        for u in range(4):
                            nc.tensor.matmul(
                                dps[:],
                                r_(qv[:, :, u]),
                                r_(wo4_sb[:, u, chd * 512:(chd + 1) * 512]),
                                start=False,
                                stop=(u == 3),
                            )
                        ds = dspool.tile([32, 512], f32, name="ds")
                        nc.scalar.activation(ds[:], dps[:], AF.Copy)
                        nc.scalar.dma_start(
                            DEC[32 * j:32 * j + 32, chd * 512:(chd + 1) * 512],
                            ds[:],
                        )

    nc.finalize()
    return nc


def _prep_shared(inputs):
    perm = _sigma_perm()
    win = np.ascontiguousarray(
        inputs["W_in"].reshape(8, 128, 256).transpose(1, 0, 2)
    ).astype(np.float32)
    bin_ = np.ascontiguousarray(inputs["b_in"].reshape(2, 128).T).astype(np.float32)

    def conv_prep(w):
        a = w.transpose(1, 2, 3, 0)                   # [ci, kh, kw, co]
        a = a.reshape(2, 128, 3, 3, 256).transpose(1, 0, 2, 3, 4)
        return np.ascontiguousarray(a.reshape(128, 4608)).astype(np.float32)

    w1p = conv_prep(inputs["conv1_w"])
    b1 = np.ascontiguousarray(inputs["conv1_b"].reshape(2, 128).T).astype(np.float32)
    w2p = conv_prep(inputs["conv2_w"][perm])          # permute OUTPUT channels
    b2 = np.ascontiguousarray(
        inputs["conv2_b"][perm].reshape(2, 128).T
    ).astype(np.float32)

    C = inputs["codebooks"].astype(np.float32)
    ct2 = np.ascontiguousarray(
        (2.0 * C.T[perm]).reshape(2, 128, 8192)
    ).astype(np.float32)
    nc2 = (-(C.astype(np.float64) ** 2).sum(1)).astype(np.float32).reshape(1, 8192)

    wo4 = np.ascontiguousarray(
        inputs["W_out"].reshape(4, 64, 1024).transpose(1, 0, 2)
    ).astype(np.float32)
    bout = inputs["b_out"].astype(np.float32).reshape(1, 1024)
    idn = np.eye(128, dtype=np.float32)
    return {
        "win": win, "bin": bin_, "w1p": w1p, "b1": b1, "w2p": w2p, "b2": b2,
        "ct2": ct2, "nc2": nc2, "wo4": wo4, "bout": bout, "idn": idn,
    }


def _prep_core(inputs, c, perm):
    first = inputs["input_data_first"][c * BC:(c + 1) * BC]   # [32, 64, 1024]
    last = inputs["input_data_last"][c * BC:(c + 1) * BC]
    xs = np.stack([first, last], axis=1)                      # [32, 2, 64, 1024]
    xT = xs.transpose(3, 0, 1, 2).reshape(1024, 4096)         # col = img2*64 + pos
    xh = np.ascontiguousarray(
        xT.reshape(8, 128, 8, 512).transpose(2, 1, 0, 3)      # [ch, p, kt, n]
    ).astype(np.float32)
    rv = inputs["random_vector"][c * RROWS:(c + 1) * RROWS]   # [128, 256]
    rtp = np.ascontiguousarray(
        rv.T[perm].reshape(2, 128, 128).transpose(1, 0, 2)
    ).astype(np.float32)
    return {"xh": xh, "rtp": rtp}


def kernel(**inputs):
    global LAST_EXEC_NS
    from concourse.bass_utils import run_bass_kernel_spmd

    inputs = {k: np.asarray(v) for k, v in inputs.items()}
    if "nc" not in _CACHE:
        _CACHE["nc"] = _build_nc()
    nc = _CACHE["nc"]

    perm = _sigma_perm()
    shared = _prep_shared(inputs)
    in_maps = []
    for c in range(NCORES):
        m = dict(shared)
        m.update(_prep_core(inputs, c, perm))
        in_maps.append(m)

    res = run_bass_kernel_spmd(
        nc, in_maps, list(range(NCORES)), trace=TRACE
    )
    LAST_EXEC_NS = res.exec_time_ns

    dec = np.empty((B, 4, DIM), np.float32)
    min_idx = np.empty((B, 4), np.int32)
    for c in range(NCORES):
        r = res.results[c]
        dec[c * BC:(c + 1) * BC] = (
            r["dec"].reshape(4, BC, DIM).transpose(1, 0, 2)
        )
        vals = r["tval"].reshape(128, 4, 8)
        idxs = r["tidx"].reshape(128, 4, 8).astype(np.int64)
        v4 = vals[:, :, 0]                     # [128, 4] chunk top-1 values
        i4 = idxs[:, :, 0]
        cstar = np.argmax(v4, axis=1)          # first max -> lowest chunk on ties
        rows = np.arange(128)
        gidx = cstar * 2048 + i4[rows, cstar]
        min_idx[c * BC:(c + 1) * BC] = gidx.reshape(BC, 4).astype(np.int32)

    flat = min_idx.reshape(-1)
    counts = np.bincount(flat, minlength=K).astype(np.float32)
    avg = (counts / np.float32(flat.shape[0])).astype(np.float32)
    perplexity = np.float32(
        np.exp(-np.sum(avg * np.log(avg + np.float32(EPS)), dtype=np.float32))
    )
    codebooks_used = np.bincount(flat, minlength=K).astype(np.int32)
    return dec, perplexity, codebooks_used, min_idx
